# revision 43
# baseline (speedup 1.0000x reference)
"""Trainium2 Bass kernel for EnergyConstrainedPredictiveCodingModel.

Data-parallel over the batch dim across 8 NeuronCores; weights replicated.

Design (DMA-bound problem; ~47 MB/core of HBM traffic at ~330 GB/s):
  * inputs + weights shipped as bf16 (PSUM accumulation stays f32); the
    l2err chain, sigma_p assembly and eps_zhat stay f32 (the output scale
    is set by l2err ~ (eps_zhat*sigma_p)^2 ~ 500, so bf16 rounding there
    is the dominant error term).
  * host-side prep: weight transposes, relu(W_vip/W_theta_to_z/b_ps),
    W_h_to_h spectral clip, W_rec2@W_rec1 fusion, 0.5*eps_z folding,
    b_prior_sigma split into bf16 hi+lo halves, and packing of all
    per-row inputs (plus pre-transposed h/h2) into one blocked buffer
    so each row tile needs two load DMAs.
  * one [128, 6656] f32 output tile assembled in SBUF per row tile,
    stored with a few column-range DMAs (early blocks stream out early).
  * PE transposes run in bf16 (full rate; f32 transposes are 1/4 rate);
    bf16 PSUM holds 8 transpose blocks per bank -> single evict.
  * wpm|wps|wi2t fused into one [D, 1536] weight so muq/sigq/ith come
    from a single 8-matmul group (3 PSUM banks, evicted in stage1).
  * activation engine only uses funcs from the exp_and_others table;
    1/(1+vip) via reciprocal_approx_fast on DVE (no act-table reloads).

Model (per reference):
  B=8192, D=1024, L=512, H=512, REC=256, MAX_NORM=0.5
  out = concat([z, h_new, h2_new, sigma_p, theta, sst_inh, theta_ff,
                z_energy, I_hat, layer_1_error, layer_2_error], -1)
"""

import numpy as np
import ml_dtypes
from contextlib import ExitStack

import concourse.bass as bass
import concourse.mybir as mybir
import concourse.tile as tile
from concourse import bacc
from concourse.bass_utils import run_bass_kernel_spmd
from concourse.masks import make_identity

B, D, L, H, REC = 8192, 1024, 512, 512, 256
MAX_NORM = 0.5
N_CORES = 8
BL = B // N_CORES            # rows per core
P = 128                      # partitions
NT = BL // P                 # row tiles per core
OUT_W = 9 * L + 2 * D        # 6656

F32 = mybir.dt.float32
BF16 = mybir.dt.bfloat16
AF = mybir.ActivationFunctionType
OP = mybir.AluOpType
BF16_NP = ml_dtypes.bfloat16

# output column offsets
OFF_Z = 0
OFF_HN = L
OFF_H2N = 2 * L
OFF_SP = 3 * L
OFF_TH = 4 * L
OFF_SST = 5 * L
OFF_TFF = 6 * L
OFF_ZE = 7 * L
OFF_IH = 8 * L
OFF_L1 = 8 * L + D
OFF_L2 = 8 * L + 2 * D

# packed input columns (bf16): it | spp | tffp | tp | sstp | eps_z/2 | hT | h2T
C_IT = 0
C_SPP = D
C_TFFP = D + L
C_TP = D + 2 * L
C_SSTP = D + 3 * L
C_EPSZ = D + 4 * L
C_HT = D + 5 * L          # 4 chunks of [128 feat, 128 rows]
C_H2T = D + 5 * L + 512   # 4 chunks
C_ITT = D + 5 * L + 1024  # 8 chunks (host-transposed I_t)
DIN_W = D + 5 * L + 2048  # 5632

# weights: name -> (K, N) of the pre-transposed [in, out] matrix
W_SHAPES = {
    "wprs": (H + P, L),      # wprs with bias hi/lo folded in as a 5th chunk
    "wpq": (D, 3 * L),       # [wpm | wps | wi2t]
    "wvip": (L, L),
    "wprm": (H, L),
    "wt2z": (L, L),
    "wzh": (L, H),
    "whh": (H, H),
    "wh2h2": (H, H),
    "wzh2": (L, H),
    "wrec": (L, D),
}


def _mm_group(nc, out_ps, lhsT_chunks, w_sb, nk, first=True, last=True,
              n_slice=None):
    """Accumulate out_ps += lhsT.T @ w over nk 128-chunks (bf16 operands)."""
    for c in range(nk):
        rhs = w_sb[:, c, :] if n_slice is None else w_sb[:, c, n_slice]
        nc.tensor.matmul(
            out_ps,
            lhsT_chunks(c),
            rhs,
            start=(first and c == 0),
            stop=(last and c == nk - 1),
        )


def _build_program(bl=BL):
    nc = bacc.Bacc(trn_type="TRN2", target_bir_lowering=False, debug=False)
    nt = bl // P

    din_d = nc.dram_tensor("din", [nt, P, DIN_W], BF16, kind="ExternalInput").ap()
    ezh_d = nc.dram_tensor("ezh", [nt, P, L], F32, kind="ExternalInput").ap()
    w_d = {
        name: nc.dram_tensor(name, [P, K // P, N], BF16, kind="ExternalInput").ap()
        for name, (K, N) in W_SHAPES.items()
    }
    out_d = nc.dram_tensor("out", [bl, OUT_W], F32, kind="ExternalOutput").ap()

    with tile.TileContext(nc) as tc, ExitStack() as ctx:
        weights = ctx.enter_context(tc.tile_pool(name="weights", bufs=1))
        consts = ctx.enter_context(tc.tile_pool(name="consts", bufs=1))
        psum = ctx.enter_context(tc.tile_pool(name="psum", bufs=5, space="PSUM"))
        pin = ctx.enter_context(tc.tile_pool(name="pin", bufs=4))
        pezh = ctx.enter_context(tc.tile_pool(name="pezh", bufs=2))
        ptr = ctx.enter_context(tc.tile_pool(name="ptr", bufs=2))
        pim = ctx.enter_context(tc.tile_pool(name="pim", bufs=2))
        pf32 = ctx.enter_context(tc.tile_pool(name="pf32", bufs=1))
        pout = ctx.enter_context(tc.tile_pool(name="pout", bufs=2))

        ident = consts.tile([P, P], BF16)
        make_identity(nc, ident)
        neg1_col = consts.tile([P, 1], F32)
        nc.vector.memset(neg1_col, -1.0)
        negh_col = consts.tile([P, 1], F32)
        nc.vector.memset(negh_col, -0.5)
        # lhsT chunk for the bias rows folded into wprs (chunk H//P):
        # rows 0/1 are all-ones so out[m,n] += wprs_ext[4][0,n] + wprs_ext[4][1,n]
        ones2 = consts.tile([P, P], BF16)
        nc.vector.memset(ones2, 0.0)
        nc.vector.memset(ones2[0:2, :], 1.0)

        # ---- prologue DMAs: first inputs, then weights in first-use order ----
        din_tiles = {}
        ezh_tiles = {}

        def load_din(t):
            din_tiles[t] = pin.tile([P, DIN_W], BF16, tag="din", name=f"din{t}")
            nc.sync.dma_start(out=din_tiles[t], in_=din_d[t])

        def load_ezh(t):
            ezh_tiles[t] = pezh.tile([P, L], F32, tag="ezh", name=f"ezh{t}")
            nc.sync.dma_start(out=ezh_tiles[t], in_=ezh_d[t])

        w_sb = {}

        def load_w(name):
            K, N = W_SHAPES[name]
            w_sb[name] = weights.tile(
                [P, K // P, N], BF16, tag=f"w_{name}", name=f"w_{name}"
            )
            nc.sync.dma_start(out=w_sb[name], in_=w_d[name])

        load_din(0)
        load_w("wprs")
        load_w("wpq")
        load_din(1)
        load_w("wvip")
        load_w("wprm")
        load_w("wt2z")
        load_din(2)
        load_ezh(0)
        load_w("whh")
        load_w("wh2h2")
        load_w("wzh")
        load_w("wzh2")
        load_w("wrec")
        load_ezh(1)

        # PE transpose src[:, :nblk*128] (bf16) -> dst [128, nblk, 128] bf16.
        # bf16 PSUM: up to 8 blocks (1024 cols = 2KB) per bank; single evict.
        def transpose_in(dst, src_cols, nblk, evict="act"):
            g = 0
            while g * 8 < nblk:
                k = min(8, nblk - g * 8)
                ps = psum.tile([P, 1024], BF16, tag="ps", name="ps_tr")
                for j in range(k):
                    blk = g * 8 + j
                    nc.tensor.transpose(
                        ps[:, j * P:(j + 1) * P],
                        src_cols[:, blk * P:(blk + 1) * P],
                        ident,
                    )
                dslice = dst[:, g * 8:g * 8 + k, :].rearrange("p c n -> p (c n)")
                if evict == "act":
                    nc.scalar.copy(dslice, ps[:, : k * P])
                else:
                    nc.vector.tensor_copy(dslice, ps[:, : k * P])
                g += 1

        # ---- software-pipelined main loop ----
        # stage1(t): transposes + matmuls/elementwise through theta; all
        #            held-PSUM groups are evicted before stage1 ends.
        # tail(t):   theta-transpose onward (sst, z, h_new, I_hat, errors)
        # Emission: S1(0), S1(1), tail(0), S1(2), tail(1), ...

        def stage1(t):
            d = din_tiles[t]
            rows = slice(t * P, (t + 1) * P)
            st = {"d": d, "rows": rows}
            if t + 3 <= nt - 1:
                load_din(t + 3)
            if t + 2 <= nt - 1:
                load_ezh(t + 2)

            ot = pout.tile([P, OUT_W], F32, tag="out", name=f"out{t}")
            st["ot"] = ot

            def hT(c):
                return d[:, C_HT + c * P:C_HT + (c + 1) * P]

            def h2T(c):
                return d[:, C_H2T + c * P:C_H2T + (c + 1) * P]

            st["hT"], st["h2T"] = hT, h2T

            # itT is host-transposed inside din
            def itT(c):
                return d[:, C_ITT + c * P:C_ITT + (c + 1) * P]

            st["itT"] = itT

            # sigma_p = 0.8*relu(h@Wprs.T + b) + 0.2*spp; the bias rides in
            # wprs chunk 4 against the ones2 stationary block
            sigp_ps = psum.tile([P, L], F32, tag="ps", name="sigp_ps")
            _mm_group(nc, sigp_ps,
                      lambda c: hT(c) if c < H // P else ones2,
                      w_sb["wprs"], H // P + 1)
            siga = pf32.tile([P, L], F32, tag="siga", name="siga")
            nc.scalar.activation(siga, sigp_ps, AF.Relu, scale=0.8)
            # critical path first: bf16 copy feeding the sigpT transpose
            sigp_b = pim.tile([P, L], BF16, tag="sigp_b", name="sigp_b")
            nc.vector.scalar_tensor_tensor(
                sigp_b, d[:, C_SPP:C_SPP + L], 0.2, siga, OP.mult, OP.add
            )
            nc.vector.scalar_tensor_tensor(
                ot[:, OFF_SP:OFF_SP + L], d[:, C_SPP:C_SPP + L], 0.2, siga,
                OP.mult, OP.add,
            )
            st["sigp_b"] = sigp_b

            # muq | sigq | ith from one fused group (held 3 banks, tag "big")
            # ith (j=2) first: the serial tff/theta chain hangs off it
            big_ps = psum.tile([P, 3 * L], F32, tag="big", name="big_ps", bufs=1)
            for j in (2, 0, 1):
                _mm_group(nc, big_ps[:, j * L:(j + 1) * L],
                          itT, w_sb["wpq"], D // P,
                          n_slice=slice(j * L, (j + 1) * L))

            # vip needs sigma_p transposed
            sigpT = ptr.tile([P, L // P, P], BF16, tag="sigpT", name="sigpT", bufs=1)
            transpose_in(sigpT, sigp_b, L // P)

            # evict the BIG group early (frees its banks for t+1's group):
            # muq and s only depend on the matmuls, not on the theta chain
            muq_b = pim.tile([P, L], BF16, tag="muq_b", name="muq_b")
            nc.scalar.activation(muq_b, big_ps[:, 0:L], AF.Relu)
            st["muq_b"] = muq_b
            # s = tanh(0.005*relu(sq)) == relu(tanh(0.005*sq)); the 0.5 of
            # (sigmoid-0.5) is folded into eps_z on the host
            s_b = pim.tile([P, L], BF16, tag="s_b", name="s_b")
            nc.scalar.activation(s_b, big_ps[:, L:2 * L], AF.Tanh, scale=0.005)
            st["s_b"] = s_b

            # theta_ff exp term first: no deps, keeps the Act queue flowing
            a1 = pim.tile([P, L], BF16, tag="a1", name="a1", bufs=1)
            nc.scalar.activation(a1, d[:, C_TFFP:C_TFFP + L], AF.Exp,
                                 scale=-50.0)

            vip_ps = psum.tile([P, L], F32, tag="ps", name="vip_ps")
            _mm_group(nc, vip_ps, lambda c: sigpT[:, c, :], w_sb["wvip"], L // P)
            # evict vip immediately (1/(1+vip) on DVE before the tf1 ops
            # that block on the BIG group) so its ring slot frees early --
            # ih0 of tail_b(t-1) allocates into it
            vip1 = pf32.tile([P, L], F32, tag="vip1", name="vip1")
            nc.vector.tensor_scalar_add(vip1, vip_ps, 1.0)
            rcp = pf32.tile([P, L], F32, tag="rcp", name="rcp")
            nc.vector.reciprocal_approx_fast(rcp, vip1)

            mup_ps = psum.tile([P, L], F32, tag="ps", name="mup_ps")
            _mm_group(nc, mup_ps, h2T, w_sb["wprm"], H // P)
            mup_f = pf32.tile([P, L], F32, tag="mup_f", name="mup_f")
            nc.scalar.activation(mup_f, mup_ps, AF.Relu)
            st["mup_f"] = mup_f

            # theta_ff = tanh(0.4*tffp + exp(-50*tffp)*ith)^2
            # (theta_ff_prev is uniform[0,1) so |tffp| == tffp)
            tf1 = pim.tile([P, L], BF16, tag="tf1", name="tf1", bufs=1)
            nc.vector.tensor_tensor(tf1, a1, big_ps[:, 2 * L:3 * L], OP.mult)
            nc.vector.scalar_tensor_tensor(
                tf1, d[:, C_TFFP:C_TFFP + L], 0.4, tf1, OP.mult, OP.add
            )
            tft = pim.tile([P, L], BF16, tag="tft", name="tft", bufs=1)
            nc.scalar.activation(tft, tf1, AF.Tanh)
            # theta_ff lands in the out tile as tanh^2 directly; the theta
            # chain reads the f32 slice
            nc.scalar.activation(ot[:, OFF_TFF:OFF_TFF + L], tft, AF.Square)
            th1 = pim.tile([P, L], BF16, tag="th1", name="th1", bufs=1)
            nc.vector.tensor_tensor(th1, ot[:, OFF_TFF:OFF_TFF + L], rcp,
                                    OP.mult)
            theta_b = pim.tile([P, L], BF16, tag="theta_b", name="theta_b")
            nc.vector.scalar_tensor_tensor(
                theta_b, d[:, C_TP:C_TP + L], 0.1, th1, OP.mult, OP.add
            )
            nc.vector.tensor_copy(ot[:, OFF_TH:OFF_TH + L], theta_b)
            st["theta_b"] = theta_b

            # early store: sigma_p + theta are final (contiguous columns);
            # stores ride the gpsimd queue so they never head-of-line-block
            # the input loads on the sync queue
            nc.gpsimd.dma_start(
                out=out_d[rows, OFF_SP:OFF_SP + 2 * L],
                in_=ot[:, OFF_SP:OFF_SP + 2 * L],
            )
            nc.gpsimd.dma_start(
                out=out_d[rows, OFF_TFF:OFF_TFF + L],
                in_=ot[:, OFF_TFF:OFF_TFF + L],
            )
            return st

        def tail_a(t, st):
            """theta-transpose, sst and the z elementwise chain: PE work is
            small; the DVE/Act chain overlaps the next stage1's matmuls."""
            d, ot = st["d"], st["ot"]
            theta_b = st["theta_b"]

            # sst_inh = 0.8*sstp + theta@Wt2z_p.T
            thetaT = ptr.tile([P, L // P, P], BF16, tag="thetaT", name="thetaT", bufs=1)
            transpose_in(thetaT, theta_b, L // P)
            sst_ps = psum.tile([P, L], F32, tag="ps", name="sst_ps")
            _mm_group(nc, sst_ps, lambda c: thetaT[:, c, :], w_sb["wt2z"], L // P)
            sst_b = pim.tile([P, L], BF16, tag="sst_b", name="sst_b")
            nc.vector.scalar_tensor_tensor(
                sst_b, d[:, C_SSTP:C_SSTP + L], 0.8, sst_ps, OP.mult, OP.add
            )
            nc.scalar.copy(ot[:, OFF_SST:OFF_SST + L], sst_b)

            # raw_z = tanh(mu_q + (eps_z/2)*relu(tanh(0.005*sq)))
            sf = pim.tile([P, L], BF16, tag="sf", name="sf", bufs=1)
            nc.vector.tensor_scalar_max(sf, st["s_b"], 0.0)
            rz = pim.tile([P, L], BF16, tag="rz", name="rz", bufs=1)
            nc.vector.tensor_tensor(rz, sf, d[:, C_EPSZ:C_EPSZ + L], OP.mult)
            nc.vector.tensor_tensor(rz, rz, st["muq_b"], OP.add)
            rzt = pim.tile([P, L], BF16, tag="rzt", name="rzt")
            nc.scalar.activation(rzt, rz, AF.Tanh)

            # z = relu(raw_z - sst)  (== z_energy)
            zd = pim.tile([P, L], BF16, tag="zd", name="zd", bufs=1)
            nc.vector.tensor_tensor(zd, rzt, sst_b, OP.subtract)
            z_b = pim.tile([P, L], BF16, tag="z_b", name="z_b")
            nc.vector.tensor_scalar_max(z_b, zd, 0.0)
            nc.scalar.copy(ot[:, OFF_Z:OFF_Z + L], z_b)
            nc.vector.tensor_copy(ot[:, OFF_ZE:OFF_ZE + L], z_b)
            st["z_b"] = z_b
            nc.gpsimd.dma_start(
                out=out_d[st["rows"], OFF_SST:OFF_SST + L],
                in_=ot[:, OFF_SST:OFF_SST + L],
            )

        def tail_b(t, st):
            d, ot, rows = st["d"], st["ot"], st["rows"]
            hT, h2T, itT = st["hT"], st["h2T"], st["itT"]
            sigp_b = st["sigp_b"]
            z_b = st["z_b"]

            # h_new / h2_new; whh/wh2h2 halves start while zT transposes
            hn_ps = psum.tile([P, H], F32, tag="ps", name="hn_ps")
            _mm_group(nc, hn_ps, hT, w_sb["whh"], H // P, last=False)
            h2n_ps = psum.tile([P, H], F32, tag="ps", name="h2n_ps")
            _mm_group(nc, h2n_ps, h2T, w_sb["wh2h2"], H // P, last=False)
            zT = ptr.tile([P, L // P, P], BF16, tag="zT", name="zT", bufs=1)
            transpose_in(zT, z_b, L // P)
            _mm_group(nc, hn_ps, lambda c: zT[:, c, :], w_sb["wzh"], L // P,
                      first=False)
            nc.scalar.activation(ot[:, OFF_HN:OFF_HN + H], hn_ps, AF.Relu)
            _mm_group(nc, h2n_ps, lambda c: zT[:, c, :], w_sb["wzh2"], L // P,
                      first=False)
            nc.scalar.activation(ot[:, OFF_H2N:OFF_H2N + H], h2n_ps, AF.Relu)

            # I_hat = sigmoid(z@W_rec.T - 2) = 0.5*tanh(0.5*(z@W_rec.T) - 1) + 0.5
            # l1 = (I_t - I_hat)^2 = ((it - 0.5*th) - 0.5)^2
            for half in range(2):
                hsl = slice(half * 512, (half + 1) * 512)
                ih_ps = psum.tile([P, 512], F32, tag="ps", name="ih_ps")
                _mm_group(nc, ih_ps, lambda c: zT[:, c, :], w_sb["wrec"],
                          L // P, n_slice=hsl)
                th_h = pim.tile([P, 512], BF16, tag="th_h", name="th_h")
                nc.scalar.activation(th_h, ih_ps, AF.Tanh, scale=0.5,
                                     bias=neg1_col)
                nc.vector.tensor_scalar(
                    ot[:, OFF_IH + half * 512:OFF_IH + (half + 1) * 512],
                    th_h, 0.5, 0.5, OP.mult, OP.add,
                )
                dh = pim.tile([P, 512], BF16, tag="dh", name="dh")
                nc.vector.scalar_tensor_tensor(
                    dh, th_h, -0.5, d[:, C_IT + half * 512:C_IT + (half + 1) * 512],
                    OP.mult, OP.add,
                )
                nc.scalar.activation(
                    ot[:, OFF_L1 + half * 512:OFF_L1 + (half + 1) * 512],
                    dh, AF.Square, bias=negh_col,
                )

            # l2 = (z - mu_p - eps_zhat*sigma_p)^2 in f32
            zh = pf32.tile([P, L], F32, tag="zh", name="zh")
            nc.vector.tensor_tensor(zh, ezh_tiles.pop(t),
                                    ot[:, OFF_SP:OFF_SP + L], OP.mult)
            d2 = pf32.tile([P, L], F32, tag="d2", name="d2")
            nc.vector.tensor_tensor(d2, ot[:, OFF_Z:OFF_Z + L], st["mup_f"],
                                    OP.subtract)
            nc.vector.tensor_tensor(d2, d2, zh, OP.subtract)
            nc.vector.tensor_tensor(ot[:, OFF_L2:OFF_L2 + L], d2, d2, OP.mult)

            # remaining stores: [z, hn, h2n], [ze, ih, l1], [l2] -- the
            # 10KB/partition ze..l1 range streams while l2 finishes
            nc.gpsimd.dma_start(
                out=out_d[rows, OFF_Z:OFF_Z + 3 * L], in_=ot[:, OFF_Z:OFF_Z + 3 * L]
            )
            nc.gpsimd.dma_start(
                out=out_d[rows, OFF_ZE:OFF_L2], in_=ot[:, OFF_ZE:OFF_L2]
            )
            nc.gpsimd.dma_start(
                out=out_d[rows, OFF_L2:OUT_W], in_=ot[:, OFF_L2:OUT_W]
            )

        # Emission: S1(0), ta(0), S1(1), tb(0), ta(1), S1(2), tb(1), ta(2), ...
        # tail_b(t-1) sits between S1(t) and tail_a(t) so the PE always has a
        # full block of matmuls between dependency stalls, and tile t's
        # DVE/Act chains overlap tile t-1's and t+1's matmul blocks.
        states = {}
        for t in range(nt):
            states[t] = stage1(t)
            if t >= 1:
                tail_b(t - 1, states[t - 1])
            tail_a(t, states[t])
        tail_b(nt - 1, states[nt - 1])

    nc.compile()
    return nc


_NC_CACHE = []


def _get_program():
    if not _NC_CACHE:
        _NC_CACHE.append(_build_program())
    return _NC_CACHE[0]


def _prep_in_maps(inputs):
    f32 = lambda a: np.asarray(a, dtype=np.float32)
    bf = lambda a: np.ascontiguousarray(np.asarray(a).astype(BF16_NP))

    # ---- packed per-row input block: [cores, NT, P, DIN_W] bf16 ----
    def rowblk(name, w):
        return f32(inputs[name]).reshape(N_CORES, NT, P, w)

    din = np.empty((N_CORES, NT, P, DIN_W), dtype=BF16_NP)
    din[..., C_IT:C_IT + D] = rowblk("I_t", D).astype(BF16_NP)
    din[..., C_SPP:C_SPP + L] = rowblk("sigma_p_prev", L).astype(BF16_NP)
    din[..., C_TFFP:C_TFFP + L] = rowblk("theta_ff_prev", L).astype(BF16_NP)
    din[..., C_TP:C_TP + L] = rowblk("theta_prev", L).astype(BF16_NP)
    din[..., C_SSTP:C_SSTP + L] = rowblk("sst_inh_prev", L).astype(BF16_NP)
    din[..., C_EPSZ:C_EPSZ + L] = (0.5 * rowblk("eps_z", L)).astype(BF16_NP)
    # hT/h2T/itT: [.., p, c*128+m] = x[.., t*128+m, c*128+p]
    for name, base, w in (("h", C_HT, H), ("h2", C_H2T, H), ("I_t", C_ITT, D)):
        hb = f32(inputs[name]).reshape(N_CORES, NT, P, w // P, P)
        hb = np.transpose(hb, (0, 1, 4, 3, 2))  # [cores, NT, p, c, m]
        din[..., base:base + w] = hb.reshape(N_CORES, NT, P, w).astype(BF16_NP)

    ezh = np.ascontiguousarray(rowblk("eps_zhat", L))

    # ---- weights: pre-transposed to [in, out], blocked [P, K//P, N] bf16 ----
    def blk(w_t):
        K, N = w_t.shape
        return bf(np.transpose(np.ascontiguousarray(w_t).reshape(K // P, P, N),
                               (1, 0, 2)))

    relu = lambda a: np.maximum(a, 0.0)
    whh_f = f32(inputs["W_h_to_h"])
    nrm = np.linalg.norm(whh_f.astype(np.float32))
    whh_f = whh_f * min(1.0, MAX_NORM / float(nrm))
    wrec_f = f32(inputs["W_rec2"]) @ f32(inputs["W_rec1"])  # [D, L]
    wpq_t = np.concatenate(
        [f32(inputs["W_post_mu"]).T, f32(inputs["W_post_sigma"]).T,
         f32(inputs["W_I_to_theta"]).T], axis=1)  # [D, 3L]

    bps_f = relu(f32(inputs["b_prior_sigma"])).reshape(1, L)
    bps_hi = bps_f.astype(BF16_NP)
    bps_lo = (bps_f - bps_hi.astype(np.float32)).astype(BF16_NP)
    # wprs extended with a bias chunk: rows H/H+1 carry bps hi/lo (they meet
    # the all-ones rows 0/1 of the ones2 stationary block)
    wprs_ext = np.zeros((H + P, L), dtype=np.float32)
    wprs_ext[:H] = f32(inputs["W_prior_sigma"]).T
    wprs_ext[H] = bps_hi.astype(np.float32)[0]
    wprs_ext[H + 1] = bps_lo.astype(np.float32)[0]

    w_host = {
        "wprs": blk(wprs_ext),
        "wpq": blk(wpq_t),
        "wvip": blk(relu(f32(inputs["W_vip"])).T),
        "wprm": blk(f32(inputs["W_prior_mu"]).T),
        "wt2z": blk(relu(f32(inputs["W_theta_to_z"])).T),
        "wzh": blk(f32(inputs["W_z_to_h"]).T),
        "whh": blk(whh_f.T),
        "wh2h2": blk(f32(inputs["W_h2_to_h2"]).T),
        "wzh2": blk(f32(inputs["W_z_to_h2"]).T),
        "wrec": blk(wrec_f.T),
    }
    return [
        {"din": np.ascontiguousarray(din[i]), "ezh": ezh[i], **w_host}
        for i in range(N_CORES)
    ]


def run(inputs, trace=False, **kw):
    nc = _get_program()
    in_maps = _prep_in_maps(inputs)
    res = run_bass_kernel_spmd(
        nc, in_maps, core_ids=list(range(N_CORES)), trace=trace, **kw
    )
    out = np.concatenate([res.results[i]["out"] for i in range(N_CORES)], axis=0)
    return out, res


def kernel(**inputs):
    out, _ = run(inputs)
    return out
